# revision 51
# baseline (speedup 1.0000x reference)
"""Distributed Trainium2 Bass kernel for the gnn_message_passing problem.

Math (reference-equivalent):
    w_raw  = sum_k etas_k^2 (Y[src,k] - Y[dst,k])^2      [E]
    w      = sqrt(relu(w_raw) + 1e-7); w = max(w, TAU)
    w      = (w > T_UP ? 0 : 1/w) + 1e-9
    deg    = segment_sum(w, dst, N)

Distribution: edges sharded across 8 NeuronCores. Per-edge feature rows are
fetched with the gpsimd dma_gather ucode. Its indices are int16, so node ids
(up to 50k) are addressed as (pair_row = node>>1) with a 2-row stride and a
parity-dependent table base; the host pre-buckets each core's edges into the
4 (src&1, dst&1) parity groups so every gather call has a fixed base.

deg: per-edge w pairs are scatter-added into a stride-256B pair table with
dma_scatter_add. The DMA's read-modify-write does NOT tolerate colliding
indices inside one call, so the host orders each group's edges by occurrence
rank (k-th edge hitting a pair row goes to round k); each (group, round)
scatter call then has all-distinct indices. Rounds ping-pong between two
tables so consecutive calls overlap; the tables are merged with an
accumulating DMA, AllReduced across the 8 cores, and written out.
"""

import numpy as np

import concourse.bacc as bacc
import concourse.bass as bass
import concourse.mybir as mybir
from concourse import library_config

F32 = mybir.dt.float32
I16 = mybir.dt.int16

N_CORES = 8
TAU = 0.1
T_UP = 5.0

# static per-(group, round) capacities in 128-token columns
DEFAULT_CAPS_COLS = (128, 56, 18, 5, 2, 1, 1, 1, 1, 1)
GROUPS = ((0, 0), (0, 1), (1, 0), (1, 1))  # (src parity, dst parity)


def build_nc(n, d, caps_cols, kt, n_cores, stage=3):
    # stage: 1 = gathers+DVE+w only, 2 = +scatter rounds, 3 = full (merge+AR)
    assert d == 64
    assert n % 2 == 0
    np2 = n // 2                      # pair rows in the Y table
    np2h = ((np2 + 1 + 63) // 64) * 64  # scatter table rows (incl. dump row)
    dump_row = np2
    gcols = sum(caps_cols)            # columns per parity group
    r_tot = 4 * gcols                 # total token columns
    e_layout = r_tot * 128
    zcols = np2h // 2                 # zero-init f32 elems per partition
    nzch = 4 if zcols % 4 == 0 else 1
    dsum = 2 * np2h                   # compacted deg length (128-divisible)
    assert dsum % 128 == 0

    # gather tiles: (group, col0-in-group, ncols)
    tiles = []
    for g in range(4):
        c = 0
        while c < gcols:
            k = min(kt, gcols - c)
            tiles.append((g, c, k))
            c += k
    nt = len(tiles)

    # scatter rounds per group: (col0-in-group, ncols); the ucode caps at
    # ~1024 indices per call, so each round is issued as <=kt-col sub-calls
    # (no ordering needed among them: indices are unique round-wide)
    rounds_by_g = []
    for g in range(4):
        base, rl = 0, []
        for cap in caps_cols:
            rl.append((base, cap))
            base += cap
        rounds_by_g.append(rl)
    rounds = [(g, b, c) for g in range(4) for (b, c) in rounds_by_g[g]]

    nc = bacc.Bacc(
        "TRN2", target_bir_lowering=False, debug=False, num_devices=n_cores
    )

    # ---- parameters -----------------------------------------------------
    y_ext = nc.declare_dram_parameter("Y", [n, d], F32, isOutput=False)
    eta_ext = nc.declare_dram_parameter("eta", [128, d], F32, isOutput=False)
    sp_ext = nc.declare_dram_parameter("srcp16", [128, e_layout // 16], I16, isOutput=False)
    dp_ext = nc.declare_dram_parameter("dstp16", [128, e_layout // 16], I16, isOutput=False)
    m0_ext = nc.declare_dram_parameter("m0", [128, r_tot], F32, isOutput=False)
    m1_ext = nc.declare_dram_parameter("m1", [128, r_tot], F32, isOutput=False)
    idx_ext = nc.declare_dram_parameter("idx16", [128, e_layout // 16], I16, isOutput=False)
    w_ext = nc.declare_dram_parameter("w_out", [128, r_tot], F32, isOutput=True)
    deg_ext = nc.declare_dram_parameter("deg_out", [n], F32, isOutput=True)

    # ---- internal DRAM --------------------------------------------------
    NTAB = 4  # scatter tables; same-table rounds serialize, so more = overlap
    tabs = [nc.dram_tensor(f"tab{i}", [np2h, 64], F32) for i in range(NTAB)]
    deg_sum = nc.dram_tensor("deg_sum", [dsum], F32)
    deg_ar = nc.dram_tensor("deg_ar", [dsum], F32, addr_space="Shared")

    # python-side replay of the scatter loop to know per-table call counts
    ncalls_fin = [0] * NTAB
    for i, (_g, _b, cap) in enumerate(rounds):
        c = 0
        while c < cap:
            ncalls_fin[i % NTAB] += 1
            c += min(kt, cap - c)

    # ---- SBUF -----------------------------------------------------------
    sp_sb = nc.alloc_sbuf_tensor("sp_sb", [128, e_layout // 16], I16)
    dp_sb = nc.alloc_sbuf_tensor("dp_sb", [128, e_layout // 16], I16)
    idx_sb = nc.alloc_sbuf_tensor("idx_sb", [128, e_layout // 16], I16)
    eta_sb = nc.alloc_sbuf_tensor("eta_sb", [128, d], F32)
    m0_sb = nc.alloc_sbuf_tensor("m0_sb", [128, r_tot], F32)
    m1_sb = nc.alloc_sbuf_tensor("m1_sb", [128, r_tot], F32)
    u_sb = [nc.alloc_sbuf_tensor(f"u_sb{i}", [128, kt, d], F32) for i in range(2)]
    v_sb = [nc.alloc_sbuf_tensor(f"v_sb{i}", [128, kt, d], F32) for i in range(2)]
    d_sb = nc.alloc_sbuf_tensor("d_sb", [128, kt, d], F32)
    de_sb = nc.alloc_sbuf_tensor("de_sb", [128, kt, d], F32)
    wraw_sb = nc.alloc_sbuf_tensor("wraw_sb", [128, r_tot], F32)
    wm_sb = nc.alloc_sbuf_tensor("wm_sb", [128, r_tot], F32)
    s_sb = nc.alloc_sbuf_tensor("s_sb", [128, r_tot], F32)
    s2_sb = nc.alloc_sbuf_tensor("s2_sb", [128, r_tot], F32)
    rcp_sb = nc.alloc_sbuf_tensor("rcp_sb", [128, r_tot], F32)
    msk_sb = nc.alloc_sbuf_tensor("msk_sb", [128, r_tot], F32)
    wfin_sb = nc.alloc_sbuf_tensor("wfin_sb", [128, r_tot], F32)
    pairs_sb = nc.alloc_sbuf_tensor("pairs_sb", [128, r_tot, 2], F32)
    zsb = nc.alloc_sbuf_tensor("zsb", [128, zcols // nzch], F32)
    bufs_sb = [
        nc.alloc_sbuf_tensor(f"buf{i}_sb", [64, dsum // 64], F32) for i in range(NTAB)
    ]

    # ---- semaphores -----------------------------------------------------
    io_sd = nc.alloc_semaphore("io_sd")
    io_eta = nc.alloc_semaphore("io_eta")
    io_m = nc.alloc_semaphore("io_m")
    io_idx = nc.alloc_semaphore("io_idx")
    z_sem = nc.alloc_semaphore("z_sem")
    zi_sem = nc.alloc_semaphore("zi_sem")
    g_sem = [nc.alloc_semaphore("g_sem0"), nc.alloc_semaphore("g_sem1")]
    v_sem = nc.alloc_semaphore("v_sem")
    vp_sem = nc.alloc_semaphore("vp_sem")
    a_sem = nc.alloc_semaphore("a_sem")
    wv_sem = nc.alloc_semaphore("wv_sem")
    pv_sem = nc.alloc_semaphore("pv_sem")
    sc_sem = [nc.alloc_semaphore(f"sc_sem{i}") for i in range(NTAB)]
    cp_sem = [nc.alloc_semaphore(f"cp_sem{i}") for i in range(NTAB)]
    ms_sem = nc.alloc_semaphore("ms_sem")
    cc_sem = nc.alloc_semaphore("cc_sem")
    do_sem = nc.alloc_semaphore("do_sem")
    do2_sem = nc.alloc_semaphore("do2_sem")
    wo_sem = nc.alloc_semaphore("wo_sem")

    y3 = y_ext[:].rearrange("(a b) d -> a b d", b=2)  # [np2, 2, 64]
    tab_flats = [
        t[:].rearrange("a b -> (a b)").rearrange("(p f) -> p f", p=128) for t in tabs
    ]

    with nc.Block() as block:

        @block.sync
        def _(sp):
            sp.dma_start(out=sp_sb[:], in_=sp_ext[:]).then_inc(io_sd, 16)
            sp.dma_start(out=dp_sb[:], in_=dp_ext[:]).then_inc(io_sd, 16)
            sp.dma_start(out=eta_sb[:], in_=eta_ext[:]).then_inc(io_eta, 16)
            sp.dma_start(out=m0_sb[:], in_=m0_ext[:]).then_inc(io_m, 16)
            sp.dma_start(out=m1_sb[:], in_=m1_ext[:]).then_inc(io_m, 16)
            sp.dma_start(out=idx_sb[:], in_=idx_ext[:]).then_inc(io_idx, 16)
            sp.wait_ge(z_sem, 1)
            zc = zcols // nzch
            for tab in tab_flats:
                for q in range(nzch):
                    sp.dma_start(
                        out=tab[:, q * zc : (q + 1) * zc], in_=zsb[:]
                    ).then_inc(zi_sem, 16)
            sp.wait_ge(wv_sem, 4)
            sp.dma_start(out=w_ext[:], in_=wfin_sb[:]).then_inc(wo_sem, 16)
            sp.wait_ge(wo_sem, 16)
            sp.wait_ge(zi_sem, 16 * NTAB * nzch)
            if stage >= 3:
                # merge: pull each table's used pair columns into SBUF
                h = np2h // 2
                for i in range(NTAB):
                    sp.wait_ge(sc_sem[i], 16 * ncalls_fin[i])
                for i in range(NTAB):
                    for q in range(2):
                        sp.dma_start(
                            out=bufs_sb[i][32 * q : 32 * (q + 1), :].rearrange(
                                "p (x b) -> p x b", b=2
                            ),
                            in_=tabs[i][q * h : (q + 1) * h, 0:2],
                        ).then_inc(cp_sem[i], 16)
                sp.wait_ge(ms_sem, 1)  # DVE summed the bufs
                sp.dma_start(
                    out=deg_sum[:].rearrange("(p f) -> p f", p=64), in_=bufs_sb[0][:]
                ).then_inc(do_sem, 16)

        @block.vector
        def _(ve):
            ve.memset(zsb[:], 0.0).then_inc(z_sem, 1)
            ve.wait_ge(io_eta, 16)
            ve.wait_ge(io_m, 32)
            for tt, (g, c0, k) in enumerate(tiles):
                b = tt % 2
                gc = g * gcols + c0
                ve.wait_ge(g_sem[b], 32 * (tt // 2 + 1))
                eta_b = eta_sb[:].unsqueeze(1).to_broadcast([128, k, d])
                ve.tensor_tensor(
                    out=d_sb[:, 0:k], in0=u_sb[b][:, 0:k], in1=v_sb[b][:, 0:k],
                    op=mybir.AluOpType.subtract,
                ).then_inc(v_sem, 1)
                ve.drain()
                ve.tensor_tensor(
                    out=de_sb[:, 0:k], in0=d_sb[:, 0:k], in1=eta_b,
                    op=mybir.AluOpType.mult,
                )
                ve.drain()
                ve.tensor_tensor(
                    out=d_sb[:, 0:k], in0=de_sb[:, 0:k], in1=de_sb[:, 0:k],
                    op=mybir.AluOpType.mult,
                )
                ve.drain()
                ve.tensor_reduce(
                    out=wraw_sb[:, gc : gc + k],
                    in_=d_sb[:, 0:k],
                    axis=mybir.AxisListType.X,
                    op=mybir.AluOpType.add,
                )
                ve.drain()
                if c0 + k == gcols:
                    # group g complete: post-chain + pairs for its columns so
                    # the scatter can start while later groups still gather
                    sl = slice(g * gcols, (g + 1) * gcols)
                    ve.tensor_scalar(
                        out=wm_sb[:, sl], in0=wraw_sb[:, sl], scalar1=0.0,
                        scalar2=1e-7, op0=mybir.AluOpType.max,
                        op1=mybir.AluOpType.add,
                    ).then_inc(vp_sem, 1)
                    ve.wait_ge(a_sem, g + 1)  # sqrt on ACT
                    ve.tensor_scalar_max(out=s2_sb[:, sl], in0=s_sb[:, sl], scalar1=TAU)
                    ve.drain()
                    ve.reciprocal(out=rcp_sb[:, sl], in_=s2_sb[:, sl])
                    ve.tensor_scalar(
                        out=msk_sb[:, sl], in0=s2_sb[:, sl], scalar1=T_UP,
                        scalar2=None, op0=mybir.AluOpType.is_le,
                    )
                    ve.drain()
                    ve.tensor_tensor(
                        out=wfin_sb[:, sl], in0=rcp_sb[:, sl], in1=msk_sb[:, sl],
                        op=mybir.AluOpType.mult,
                    )
                    ve.drain()
                    ve.scalar_tensor_tensor(
                        out=pairs_sb[:, sl, 0], in0=wfin_sb[:, sl], scalar=1e-9,
                        op0=mybir.AluOpType.add, op1=mybir.AluOpType.mult,
                        in1=m0_sb[:, sl],
                    )
                    ve.scalar_tensor_tensor(
                        out=pairs_sb[:, sl, 1], in0=wfin_sb[:, sl], scalar=1e-9,
                        op0=mybir.AluOpType.add, op1=mybir.AluOpType.mult,
                        in1=m1_sb[:, sl],
                    ).then_inc(pv_sem, 1)
                    ve.drain()
                    ve.tensor_scalar_add(
                        out=wfin_sb[:, sl], in0=wfin_sb[:, sl], scalar1=1e-9
                    ).then_inc(wv_sem, 1)
            if stage >= 3:
                for i in range(NTAB):
                    ve.wait_ge(cp_sem[i], 32)
                for i in range(1, NTAB):
                    ins = ve.tensor_tensor(
                        out=bufs_sb[0][:], in0=bufs_sb[0][:], in1=bufs_sb[i][:],
                        op=mybir.AluOpType.add,
                    )
                    if i < NTAB - 1:
                        ve.drain()
                ins.then_inc(ms_sem, 1)

        @block.scalar
        def _(ac):
            for g in range(4):
                sl = slice(g * gcols, (g + 1) * gcols)
                ac.wait_ge(vp_sem, g + 1)
                ac.sqrt(s_sb[:, sl], wm_sb[:, sl]).then_inc(a_sem, 1)

        @block.gpsimd
        def _(gp):
            ncalls = [0] * NTAB
            round_i = [0]

            def scat_group(q):
                # issue group q's scatter rounds (pairs must be ready)
                if round_i[0] == 0:
                    gp.wait_ge(io_idx, 16)
                    gp.wait_ge(zi_sem, 16 * NTAB * nzch)
                gp.wait_ge(pv_sem, q + 1)
                for base, cap in rounds_by_g[q]:
                    p = round_i[0] % NTAB
                    round_i[0] += 1
                    if ncalls[p] > 0:
                        # serialize against all prior same-table rounds
                        gp.wait_ge(sc_sem[p], 16 * ncalls[p])
                    c = 0
                    while c < cap:
                        ck = min(kt, cap - c)
                        gc = q * gcols + base + c
                        ntok = ck * 128
                        gp.dma_scatter_add(
                            tabs[p][:, 0:2],
                            pairs_sb[:, gc : gc + ck, :],
                            idx_sb[:, gc * 8 : gc * 8 + ntok // 16],
                            ntok, ntok, 2, elem_step=64,
                        ).then_inc(sc_sem[p], 16)
                        ncalls[p] += 1
                        c += ck

            gp.load_library(library_config.mlp)
            gp.wait_ge(io_sd, 32)
            for tt, (g, c0, k) in enumerate(tiles):
                if stage >= 2 and c0 == 0 and g >= 2:
                    scat_group(g - 2)  # overlap: scatter g-2 while g gathers
                if tt >= 2:
                    gp.wait_ge(v_sem, tt - 1)
                b = tt % 2
                bu, bv = GROUPS[g]
                gc = g * gcols + c0
                ntok = k * 128
                gp.dma_gather(
                    u_sb[b][:, 0:k], y3[:, bu, :],
                    sp_sb[:, gc * 8 : gc * 8 + ntok // 16],
                    ntok, ntok, d, elem_step=2 * d,
                ).then_inc(g_sem[b], 16)
                gp.dma_gather(
                    v_sb[b][:, 0:k], y3[:, bv, :],
                    dp_sb[:, gc * 8 : gc * 8 + ntok // 16],
                    ntok, ntok, d, elem_step=2 * d,
                ).then_inc(g_sem[b], 16)
            if stage < 2:
                gp.wait_ge(pv_sem, 4)
                gp.dma_start(out=deg_ext[:], in_=deg_ar[0:n]).then_inc(do_sem, 16)
                gp.wait_ge(do_sem, 16)
                return
            scat_group(2)
            scat_group(3)
            assert ncalls == ncalls_fin, (ncalls, ncalls_fin)
            if stage < 3:
                for p in range(NTAB):
                    gp.wait_ge(sc_sem[p], 16 * ncalls[p])
                gp.dma_start(out=deg_ext[:], in_=deg_ar[0:n]).then_inc(do_sem, 16)
                gp.wait_ge(do_sem, 16)
                return
            gp.wait_ge(do_sem, 16)  # S wrote deg_sum
            gp.collective_compute(
                "AllReduce",
                mybir.AluOpType.add,
                replica_groups=[list(range(n_cores))],
                ins=[deg_sum[:]],
                outs=[deg_ar[:]],
            ).then_inc(cc_sem, 1)
            gp.wait_ge(cc_sem, 1)
            gp.dma_start(out=deg_ext[:], in_=deg_ar[0:n]).then_inc(do2_sem, 16)
            gp.wait_ge(do2_sem, 16)

    nc.compile()
    return nc


def _assign_rounds(rows, caps_tok):
    """Round index per token; round r gets each pair row at most once."""
    order = np.argsort(rows, kind="stable")
    sr = rows[order]
    first = np.searchsorted(sr, sr)
    rank = np.arange(len(sr)) - first
    rounds = np.empty(len(rows), np.int64)
    rounds[order] = rank
    sizes = np.bincount(rounds, minlength=len(caps_tok))
    if len(sizes) > len(caps_tok) or (sizes > np.asarray(caps_tok)).any():
        raise ValueError(
            f"round capacities exceeded: sizes={sizes.tolist()} caps={caps_tok}"
        )
    return rounds


def make_in_maps(Y, etas, src, dst, n, e_core, caps_cols, n_cores):
    np2 = n // 2
    dump_row = np2
    gcols = sum(caps_cols)
    r_tot = 4 * gcols
    e_layout = r_tot * 128
    caps_tok = [c * 128 for c in caps_cols]
    round_base = np.concatenate([[0], np.cumsum(caps_tok)[:-1]])

    eta_rep = np.ascontiguousarray(np.tile(etas[None, :], (128, 1)), dtype=np.float32)
    wrap16 = lambda a: np.tile(np.ascontiguousarray(a.reshape(-1, 16).T), (8, 1))
    wrap128 = lambda a: np.ascontiguousarray(a.reshape(-1, 128).T)

    in_maps = []
    slot_maps = []
    for c in range(n_cores):
        s = src[c * e_core : (c + 1) * e_core].astype(np.int64)
        t = dst[c * e_core : (c + 1) * e_core].astype(np.int64)
        srcp = np.zeros(e_layout, np.int16)
        dstp = np.zeros(e_layout, np.int16)
        pidx = np.full(e_layout, dump_row, np.int16)
        mm0 = np.zeros(e_layout, np.float32)
        mm1 = np.zeros(e_layout, np.float32)
        wslot = np.full(e_layout, -1, np.int64)
        for g, (bu, bv) in enumerate(GROUPS):
            sel = np.where(((s & 1) == bu) & ((t & 1) == bv))[0]
            rows = t[sel] >> 1
            rnd = _assign_rounds(rows, caps_tok)
            for r in range(len(caps_tok)):
                tok = sel[rnd == r]
                if len(tok) == 0:
                    continue
                base = g * gcols * 128 + round_base[r]
                slots = base + np.arange(len(tok))
                srcp[slots] = (s[tok] >> 1).astype(np.int16)
                dstp[slots] = (t[tok] >> 1).astype(np.int16)
                pidx[slots] = (t[tok] >> 1).astype(np.int16)
                mm0[slots] = 1.0 - bv
                mm1[slots] = float(bv)
                wslot[slots] = tok
        in_maps.append(
            {
                "Y": np.ascontiguousarray(Y, dtype=np.float32),
                "eta": eta_rep,
                "srcp16": wrap16(srcp),
                "dstp16": wrap16(dstp),
                "m0": wrap128(mm0),
                "m1": wrap128(mm1),
                "idx16": wrap16(pidx),
            }
        )
        slot_maps.append(wslot)
    return in_maps, slot_maps


_NC_CACHE = {}
LAST_EXEC_NS = None


def _ensure_ntff_hook():
    """Provide antenv.axon_hooks (absent on this image) so trace=True works."""
    import sys
    import types

    try:
        from antenv.axon_hooks import get_axon_ntff_profile_hook  # noqa: F401
        return
    except ImportError:
        pass
    import antenv
    from trn_agent_boot.trn_boot import _ntff_profile_via_ctypes

    mod = types.ModuleType("antenv.axon_hooks")
    mod._hook = _ntff_profile_via_ctypes("/opt/axon/libaxon_pjrt.so")
    mod.get_axon_ntff_profile_hook = lambda: mod._hook
    mod.set_axon_ntff_profile_hook = lambda h: setattr(mod, "_hook", h)
    sys.modules["antenv.axon_hooks"] = mod
    antenv.axon_hooks = mod

    # keep artifacts local; the default uploads to a share we don't have
    from concourse import bass_utils as _bu
    _bu.upload_artifacts = lambda tmpdir: tmpdir


def kernel(Y, etas, src, dst, _trace=False):
    global LAST_EXEC_NS
    from concourse.bass_utils import run_bass_kernel_spmd

    if _trace:
        _ensure_ntff_hook()

    Y = np.asarray(Y)
    etas = np.asarray(etas)
    src = np.asarray(src)
    dst = np.asarray(dst)
    n, d = Y.shape
    e_total = src.shape[0]
    assert e_total % N_CORES == 0
    e_core = e_total // N_CORES
    caps_cols = DEFAULT_CAPS_COLS

    in_maps, slot_maps = make_in_maps(
        Y, etas, src, dst, n, e_core, caps_cols, N_CORES
    )

    key = (n, d, caps_cols)
    if key not in _NC_CACHE:
        _NC_CACHE[key] = build_nc(n, d, caps_cols, 8, N_CORES)
    nc = _NC_CACHE[key]

    res = run_bass_kernel_spmd(
        nc, in_maps, core_ids=list(range(N_CORES)), trace=_trace
    )
    LAST_EXEC_NS = res.exec_time_ns

    w = np.empty(e_total, np.float32)
    for c in range(N_CORES):
        w_tok = res.results[c]["w_out"].reshape(128, -1).T.reshape(-1)
        wslot = slot_maps[c]
        m = wslot >= 0
        w[c * e_core + wslot[m]] = w_tok[m]
    deg = res.results[0]["deg_out"].reshape(-1).astype(np.float32)
    return w, deg


# revision 55
# speedup vs baseline: 1.2032x; 1.2032x over previous
"""Distributed Trainium2 Bass kernel for the gnn_message_passing problem.

Math (reference-equivalent):
    w_raw  = sum_k etas_k^2 (Y[src,k] - Y[dst,k])^2      [E]
    w      = sqrt(relu(w_raw) + 1e-7); w = max(w, TAU)
    w      = (w > T_UP ? 0 : 1/w) + 1e-9
    deg    = segment_sum(w, dst, N)

Distribution: edges sharded across 8 NeuronCores. Per-edge feature rows are
fetched with the gpsimd dma_gather ucode. Its indices are int16, so node ids
(up to 50k) are addressed as (pair_row = node>>1) with a 2-row stride and a
parity-dependent table base; the host pre-buckets each core's edges into the
4 (src&1, dst&1) parity groups so every gather call has a fixed base.

deg: per-edge w pairs are scatter-added into a stride-256B pair table with
dma_scatter_add. The DMA's read-modify-write does NOT tolerate colliding
indices inside one call, so the host orders each group's edges by occurrence
rank (k-th edge hitting a pair row goes to round k); each (group, round)
scatter call then has all-distinct indices. Rounds ping-pong between two
tables so consecutive calls overlap; the tables are merged with an
accumulating DMA, AllReduced across the 8 cores, and written out.
"""

import numpy as np

import concourse.bacc as bacc
import concourse.bass as bass
import concourse.mybir as mybir
from concourse import library_config

F32 = mybir.dt.float32
I16 = mybir.dt.int16

N_CORES = 8
TAU = 0.1
T_UP = 5.0

# static per-(group, round) capacities in 128-token columns
DEFAULT_CAPS_COLS = (126, 53, 17, 5, 2, 1, 1, 1, 1, 1)
GROUPS = ((0, 0), (0, 1), (1, 0), (1, 1))  # (src parity, dst parity)


def build_nc(n, d, caps_cols, kt, n_cores, stage=3):
    # stage: 1 = gathers+DVE+w only, 2 = +scatter rounds, 3 = full (merge+AR)
    assert d == 64
    assert n % 2 == 0
    np2 = n // 2                      # pair rows in the Y table
    np2h = ((np2 + 1 + 63) // 64) * 64  # scatter table rows (incl. dump row)
    dump_row = np2
    gcols = sum(caps_cols)            # columns per parity group
    r_tot = 4 * gcols                 # total token columns
    e_layout = r_tot * 128
    zcols = np2h // 2                 # zero-init f32 elems per partition
    nzch = 4 if zcols % 4 == 0 else 1
    dsum = 2 * np2h                   # compacted deg length (128-divisible)
    assert dsum % 128 == 0

    # gather tiles: (group, col0-in-group, ncols)
    tiles = []
    for g in range(4):
        c = 0
        while c < gcols:
            k = min(kt, gcols - c)
            tiles.append((g, c, k))
            c += k
    nt = len(tiles)

    # scatter rounds per group: (col0-in-group, ncols); the ucode caps at
    # ~1024 indices per call, so each round is issued as <=kt-col sub-calls
    # (no ordering needed among them: indices are unique round-wide)
    rounds_by_g = []
    for g in range(4):
        base, rl = 0, []
        for cap in caps_cols:
            rl.append((base, cap))
            base += cap
        rounds_by_g.append(rl)
    rounds = [(g, b, c) for g in range(4) for (b, c) in rounds_by_g[g]]

    nc = bacc.Bacc(
        "TRN2", target_bir_lowering=False, debug=False, num_devices=n_cores
    )

    # ---- parameters -----------------------------------------------------
    y_ext = nc.declare_dram_parameter("Y", [n, d], F32, isOutput=False)
    eta_ext = nc.declare_dram_parameter("eta", [128, d], F32, isOutput=False)
    sp_ext = nc.declare_dram_parameter("srcp16", [128, e_layout // 16], I16, isOutput=False)
    dp_ext = nc.declare_dram_parameter("dstp16", [128, e_layout // 16], I16, isOutput=False)
    m0_ext = nc.declare_dram_parameter("m0", [128, r_tot], F32, isOutput=False)
    m1_ext = nc.declare_dram_parameter("m1", [128, r_tot], F32, isOutput=False)
    idx_ext = nc.declare_dram_parameter("idx16", [128, e_layout // 16], I16, isOutput=False)
    w_ext = nc.declare_dram_parameter("w_out", [128, r_tot], F32, isOutput=True)
    deg_ext = nc.declare_dram_parameter("deg_out", [n], F32, isOutput=True)

    # ---- internal DRAM --------------------------------------------------
    # scatter tables; same-table rounds serialize on full DMA drains, so the
    # four big rank-0 rounds get dedicated tables 0-3 (never reused -> no
    # waits) and the small rounds cycle tables 4-11 (8-deep, effectively
    # wait-free too)
    NTAB = 12
    tabs = [nc.dram_tensor(f"tab{i}", [np2h, 64], F32) for i in range(NTAB)]
    deg_sum = nc.dram_tensor("deg_sum", [dsum], F32)
    deg_ar = nc.dram_tensor("deg_ar", [dsum], F32, addr_space="Shared")

    nrpg = len(caps_cols)
    table_seq = []
    _sc = 0
    for g in range(4):
        for j in range(nrpg):
            if j == 0:
                table_seq.append(g)
            else:
                table_seq.append(4 + _sc % 8)
                _sc += 1

    # python-side replay of the scatter loop to know per-table call counts
    ncalls_fin = [0] * NTAB
    for i, (_g, _b, cap) in enumerate(rounds):
        c = 0
        while c < cap:
            ncalls_fin[table_seq[i]] += 1
            c += min(kt, cap - c)

    # ---- SBUF -----------------------------------------------------------
    sp_sb = nc.alloc_sbuf_tensor("sp_sb", [128, e_layout // 16], I16)
    dp_sb = nc.alloc_sbuf_tensor("dp_sb", [128, e_layout // 16], I16)
    idx_sb = nc.alloc_sbuf_tensor("idx_sb", [128, e_layout // 16], I16)
    eta_sb = nc.alloc_sbuf_tensor("eta_sb", [128, d], F32)
    m0_sb = nc.alloc_sbuf_tensor("m0_sb", [128, r_tot], F32)
    m1_sb = nc.alloc_sbuf_tensor("m1_sb", [128, r_tot], F32)
    u_sb = [nc.alloc_sbuf_tensor(f"u_sb{i}", [128, kt, d], F32) for i in range(2)]
    v_sb = [nc.alloc_sbuf_tensor(f"v_sb{i}", [128, kt, d], F32) for i in range(2)]
    d_sb = nc.alloc_sbuf_tensor("d_sb", [128, kt, d], F32)
    de_sb = nc.alloc_sbuf_tensor("de_sb", [128, kt, d], F32)
    wraw_sb = nc.alloc_sbuf_tensor("wraw_sb", [128, r_tot], F32)
    wm_sb = nc.alloc_sbuf_tensor("wm_sb", [128, r_tot], F32)
    s_sb = nc.alloc_sbuf_tensor("s_sb", [128, r_tot], F32)
    s2_sb = nc.alloc_sbuf_tensor("s2_sb", [128, r_tot], F32)
    rcp_sb = nc.alloc_sbuf_tensor("rcp_sb", [128, r_tot], F32)
    msk_sb = nc.alloc_sbuf_tensor("msk_sb", [128, r_tot], F32)
    wfin_sb = nc.alloc_sbuf_tensor("wfin_sb", [128, r_tot], F32)
    pairs_sb = nc.alloc_sbuf_tensor("pairs_sb", [128, r_tot, 2], F32)
    zsb = nc.alloc_sbuf_tensor("zsb", [128, zcols // nzch], F32)
    bufs_sb = [
        nc.alloc_sbuf_tensor(f"buf{i}_sb", [64, dsum // 64], F32) for i in range(NTAB)
    ]

    # ---- semaphores -----------------------------------------------------
    io_sd = nc.alloc_semaphore("io_sd")
    io_eta = nc.alloc_semaphore("io_eta")
    io_m = nc.alloc_semaphore("io_m")
    io_idx = nc.alloc_semaphore("io_idx")
    z_sem = nc.alloc_semaphore("z_sem")
    zi_sem = nc.alloc_semaphore("zi_sem")
    g_sem = [nc.alloc_semaphore("g_sem0"), nc.alloc_semaphore("g_sem1")]
    v_sem = nc.alloc_semaphore("v_sem")
    vp_sem = nc.alloc_semaphore("vp_sem")
    a_sem = nc.alloc_semaphore("a_sem")
    wv_sem = nc.alloc_semaphore("wv_sem")
    pv_sem = nc.alloc_semaphore("pv_sem")
    sc_sem = [nc.alloc_semaphore(f"sc_sem{i}") for i in range(NTAB)]
    cp_sem = [nc.alloc_semaphore(f"cp_sem{i}") for i in range(NTAB)]
    ms_sem = nc.alloc_semaphore("ms_sem")
    cc_sem = nc.alloc_semaphore("cc_sem")
    do_sem = nc.alloc_semaphore("do_sem")
    do2_sem = nc.alloc_semaphore("do2_sem")
    wo_sem = nc.alloc_semaphore("wo_sem")

    y3 = y_ext[:].rearrange("(a b) d -> a b d", b=2)  # [np2, 2, 64]
    tab_flats = [
        t[:].rearrange("a b -> (a b)").rearrange("(p f) -> p f", p=128) for t in tabs
    ]

    with nc.Block() as block:

        @block.sync
        def _(sp):
            sp.dma_start(out=sp_sb[:], in_=sp_ext[:]).then_inc(io_sd, 16)
            sp.dma_start(out=dp_sb[:], in_=dp_ext[:]).then_inc(io_sd, 16)
            sp.dma_start(out=eta_sb[:], in_=eta_ext[:]).then_inc(io_eta, 16)
            sp.dma_start(out=m0_sb[:], in_=m0_ext[:]).then_inc(io_m, 16)
            sp.dma_start(out=m1_sb[:], in_=m1_ext[:]).then_inc(io_m, 16)
            sp.dma_start(out=idx_sb[:], in_=idx_ext[:]).then_inc(io_idx, 16)
            sp.wait_ge(z_sem, 1)
            zc = zcols // nzch
            for tab in tab_flats:
                for q in range(nzch):
                    sp.dma_start(
                        out=tab[:, q * zc : (q + 1) * zc], in_=zsb[:]
                    ).then_inc(zi_sem, 16)
            sp.wait_ge(wv_sem, 4)
            sp.dma_start(out=w_ext[:], in_=wfin_sb[:]).then_inc(wo_sem, 16)
            sp.wait_ge(wo_sem, 16)
            sp.wait_ge(zi_sem, 16 * NTAB * nzch)
            if stage >= 3:
                # merge: pull each table's used pair columns into SBUF
                h = np2h // 2
                for i in range(NTAB):
                    sp.wait_ge(sc_sem[i], 16 * ncalls_fin[i])
                for i in range(NTAB):
                    for q in range(2):
                        sp.dma_start(
                            out=bufs_sb[i][32 * q : 32 * (q + 1), :].rearrange(
                                "p (x b) -> p x b", b=2
                            ),
                            in_=tabs[i][q * h : (q + 1) * h, 0:2],
                        ).then_inc(cp_sem[i], 16)
                sp.wait_ge(ms_sem, 1)  # DVE summed the bufs
                sp.dma_start(
                    out=deg_sum[:].rearrange("(p f) -> p f", p=64), in_=bufs_sb[0][:]
                ).then_inc(do_sem, 16)

        @block.vector
        def _(ve):
            ve.memset(zsb[:], 0.0).then_inc(z_sem, 1)
            ve.wait_ge(io_eta, 16)
            ve.wait_ge(io_m, 32)
            for tt, (g, c0, k) in enumerate(tiles):
                b = tt % 2
                gc = g * gcols + c0
                ve.wait_ge(g_sem[b], 32 * (tt // 2 + 1))
                eta_b = eta_sb[:].unsqueeze(1).to_broadcast([128, k, d])
                ve.tensor_tensor(
                    out=d_sb[:, 0:k], in0=u_sb[b][:, 0:k], in1=v_sb[b][:, 0:k],
                    op=mybir.AluOpType.subtract,
                ).then_inc(v_sem, 1)
                ve.drain()
                ve.tensor_tensor(
                    out=de_sb[:, 0:k], in0=d_sb[:, 0:k], in1=eta_b,
                    op=mybir.AluOpType.mult,
                )
                ve.drain()
                ve.tensor_tensor(
                    out=d_sb[:, 0:k], in0=de_sb[:, 0:k], in1=de_sb[:, 0:k],
                    op=mybir.AluOpType.mult,
                )
                ve.drain()
                ve.tensor_reduce(
                    out=wraw_sb[:, gc : gc + k],
                    in_=d_sb[:, 0:k],
                    axis=mybir.AxisListType.X,
                    op=mybir.AluOpType.add,
                )
                ve.drain()
                if c0 + k == gcols:
                    # group g complete: post-chain + pairs for its columns so
                    # the scatter can start while later groups still gather
                    sl = slice(g * gcols, (g + 1) * gcols)
                    ve.tensor_scalar(
                        out=wm_sb[:, sl], in0=wraw_sb[:, sl], scalar1=0.0,
                        scalar2=1e-7, op0=mybir.AluOpType.max,
                        op1=mybir.AluOpType.add,
                    ).then_inc(vp_sem, 1)
                    ve.wait_ge(a_sem, g + 1)  # sqrt on ACT
                    ve.tensor_scalar_max(out=s2_sb[:, sl], in0=s_sb[:, sl], scalar1=TAU)
                    ve.drain()
                    ve.reciprocal(out=rcp_sb[:, sl], in_=s2_sb[:, sl])
                    ve.tensor_scalar(
                        out=msk_sb[:, sl], in0=s2_sb[:, sl], scalar1=T_UP,
                        scalar2=None, op0=mybir.AluOpType.is_le,
                    )
                    ve.drain()
                    ve.tensor_tensor(
                        out=wfin_sb[:, sl], in0=rcp_sb[:, sl], in1=msk_sb[:, sl],
                        op=mybir.AluOpType.mult,
                    )
                    ve.drain()
                    ve.scalar_tensor_tensor(
                        out=pairs_sb[:, sl, 0], in0=wfin_sb[:, sl], scalar=1e-9,
                        op0=mybir.AluOpType.add, op1=mybir.AluOpType.mult,
                        in1=m0_sb[:, sl],
                    )
                    ve.scalar_tensor_tensor(
                        out=pairs_sb[:, sl, 1], in0=wfin_sb[:, sl], scalar=1e-9,
                        op0=mybir.AluOpType.add, op1=mybir.AluOpType.mult,
                        in1=m1_sb[:, sl],
                    ).then_inc(pv_sem, 1)
                    ve.drain()
                    ve.tensor_scalar_add(
                        out=wfin_sb[:, sl], in0=wfin_sb[:, sl], scalar1=1e-9
                    ).then_inc(wv_sem, 1)
            if stage >= 3:
                for i in range(NTAB):
                    ve.wait_ge(cp_sem[i], 32)
                for i in range(1, NTAB):
                    ins = ve.tensor_tensor(
                        out=bufs_sb[0][:], in0=bufs_sb[0][:], in1=bufs_sb[i][:],
                        op=mybir.AluOpType.add,
                    )
                    if i < NTAB - 1:
                        ve.drain()
                ins.then_inc(ms_sem, 1)

        @block.scalar
        def _(ac):
            for g in range(4):
                sl = slice(g * gcols, (g + 1) * gcols)
                ac.wait_ge(vp_sem, g + 1)
                ac.sqrt(s_sb[:, sl], wm_sb[:, sl]).then_inc(a_sem, 1)

        @block.gpsimd
        def _(gp):
            ncalls = [0] * NTAB
            round_i = [0]

            def scat_group(q):
                # issue group q's scatter rounds (pairs must be ready)
                if round_i[0] == 0:
                    gp.wait_ge(io_idx, 16)
                    gp.wait_ge(zi_sem, 16 * NTAB * nzch)
                gp.wait_ge(pv_sem, q + 1)
                for base, cap in rounds_by_g[q]:
                    p = table_seq[round_i[0]]
                    round_i[0] += 1
                    if ncalls[p] > 0:
                        # serialize against all prior same-table rounds
                        gp.wait_ge(sc_sem[p], 16 * ncalls[p])
                    c = 0
                    while c < cap:
                        ck = min(kt, cap - c)
                        gc = q * gcols + base + c
                        ntok = ck * 128
                        gp.dma_scatter_add(
                            tabs[p][:, 0:2],
                            pairs_sb[:, gc : gc + ck, :],
                            idx_sb[:, gc * 8 : gc * 8 + ntok // 16],
                            ntok, ntok, 2, elem_step=64,
                        ).then_inc(sc_sem[p], 16)
                        ncalls[p] += 1
                        c += ck

            gp.load_library(library_config.mlp)
            gp.wait_ge(io_sd, 32)
            for tt, (g, c0, k) in enumerate(tiles):
                if stage >= 2 and c0 == 0 and g >= 2:
                    scat_group(g - 2)  # overlap: scatter g-2 while g gathers
                if tt >= 2:
                    gp.wait_ge(v_sem, tt - 1)
                b = tt % 2
                bu, bv = GROUPS[g]
                gc = g * gcols + c0
                ntok = k * 128
                gp.dma_gather(
                    u_sb[b][:, 0:k], y3[:, bu, :],
                    sp_sb[:, gc * 8 : gc * 8 + ntok // 16],
                    ntok, ntok, d, elem_step=2 * d,
                ).then_inc(g_sem[b], 16)
                gp.dma_gather(
                    v_sb[b][:, 0:k], y3[:, bv, :],
                    dp_sb[:, gc * 8 : gc * 8 + ntok // 16],
                    ntok, ntok, d, elem_step=2 * d,
                ).then_inc(g_sem[b], 16)
            if stage < 2:
                gp.wait_ge(pv_sem, 4)
                gp.dma_start(out=deg_ext[:], in_=deg_ar[0:n]).then_inc(do_sem, 16)
                gp.wait_ge(do_sem, 16)
                return
            scat_group(2)
            scat_group(3)
            assert ncalls == ncalls_fin, (ncalls, ncalls_fin)
            if stage < 3:
                for p in range(NTAB):
                    gp.wait_ge(sc_sem[p], 16 * ncalls[p])
                gp.dma_start(out=deg_ext[:], in_=deg_ar[0:n]).then_inc(do_sem, 16)
                gp.wait_ge(do_sem, 16)
                return
            gp.wait_ge(do_sem, 16)  # S wrote deg_sum
            gp.collective_compute(
                "AllReduce",
                mybir.AluOpType.add,
                replica_groups=[list(range(n_cores))],
                ins=[deg_sum[:]],
                outs=[deg_ar[:]],
            ).then_inc(cc_sem, 1)
            gp.wait_ge(cc_sem, 1)
            gp.dma_start(out=deg_ext[:], in_=deg_ar[0:n]).then_inc(do2_sem, 16)
            gp.wait_ge(do2_sem, 16)

    nc.compile()
    return nc


def _assign_rounds(rows, caps_tok):
    """Round index per token; round r gets each pair row at most once."""
    order = np.argsort(rows, kind="stable")
    sr = rows[order]
    first = np.searchsorted(sr, sr)
    rank = np.arange(len(sr)) - first
    rounds = np.empty(len(rows), np.int64)
    rounds[order] = rank
    sizes = np.bincount(rounds, minlength=len(caps_tok))
    if len(sizes) > len(caps_tok) or (sizes > np.asarray(caps_tok)).any():
        raise ValueError(
            f"round capacities exceeded: sizes={sizes.tolist()} caps={caps_tok}"
        )
    return rounds


def make_in_maps(Y, etas, src, dst, n, e_core, caps_cols, n_cores):
    np2 = n // 2
    dump_row = np2
    gcols = sum(caps_cols)
    r_tot = 4 * gcols
    e_layout = r_tot * 128
    caps_tok = [c * 128 for c in caps_cols]
    round_base = np.concatenate([[0], np.cumsum(caps_tok)[:-1]])

    eta_rep = np.ascontiguousarray(np.tile(etas[None, :], (128, 1)), dtype=np.float32)
    wrap16 = lambda a: np.tile(np.ascontiguousarray(a.reshape(-1, 16).T), (8, 1))
    wrap128 = lambda a: np.ascontiguousarray(a.reshape(-1, 128).T)

    in_maps = []
    slot_maps = []
    for c in range(n_cores):
        s = src[c * e_core : (c + 1) * e_core].astype(np.int64)
        t = dst[c * e_core : (c + 1) * e_core].astype(np.int64)
        srcp = np.zeros(e_layout, np.int16)
        dstp = np.zeros(e_layout, np.int16)
        pidx = np.full(e_layout, dump_row, np.int16)
        mm0 = np.zeros(e_layout, np.float32)
        mm1 = np.zeros(e_layout, np.float32)
        wslot = np.full(e_layout, -1, np.int64)
        for g, (bu, bv) in enumerate(GROUPS):
            sel = np.where(((s & 1) == bu) & ((t & 1) == bv))[0]
            rows = t[sel] >> 1
            rnd = _assign_rounds(rows, caps_tok)
            for r in range(len(caps_tok)):
                tok = sel[rnd == r]
                if len(tok) == 0:
                    continue
                base = g * gcols * 128 + round_base[r]
                slots = base + np.arange(len(tok))
                srcp[slots] = (s[tok] >> 1).astype(np.int16)
                dstp[slots] = (t[tok] >> 1).astype(np.int16)
                pidx[slots] = (t[tok] >> 1).astype(np.int16)
                mm0[slots] = 1.0 - bv
                mm1[slots] = float(bv)
                wslot[slots] = tok
        in_maps.append(
            {
                "Y": np.ascontiguousarray(Y, dtype=np.float32),
                "eta": eta_rep,
                "srcp16": wrap16(srcp),
                "dstp16": wrap16(dstp),
                "m0": wrap128(mm0),
                "m1": wrap128(mm1),
                "idx16": wrap16(pidx),
            }
        )
        slot_maps.append(wslot)
    return in_maps, slot_maps


_NC_CACHE = {}
LAST_EXEC_NS = None


def _ensure_ntff_hook():
    """Provide antenv.axon_hooks (absent on this image) so trace=True works."""
    import sys
    import types

    try:
        from antenv.axon_hooks import get_axon_ntff_profile_hook  # noqa: F401
        return
    except ImportError:
        pass
    import antenv
    from trn_agent_boot.trn_boot import _ntff_profile_via_ctypes

    mod = types.ModuleType("antenv.axon_hooks")
    mod._hook = _ntff_profile_via_ctypes("/opt/axon/libaxon_pjrt.so")
    mod.get_axon_ntff_profile_hook = lambda: mod._hook
    mod.set_axon_ntff_profile_hook = lambda h: setattr(mod, "_hook", h)
    sys.modules["antenv.axon_hooks"] = mod
    antenv.axon_hooks = mod

    # keep artifacts local; the default uploads to a share we don't have
    from concourse import bass_utils as _bu
    _bu.upload_artifacts = lambda tmpdir: tmpdir


def kernel(Y, etas, src, dst, _trace=False):
    global LAST_EXEC_NS
    from concourse.bass_utils import run_bass_kernel_spmd

    if _trace:
        _ensure_ntff_hook()

    Y = np.asarray(Y)
    etas = np.asarray(etas)
    src = np.asarray(src)
    dst = np.asarray(dst)
    n, d = Y.shape
    e_total = src.shape[0]
    assert e_total % N_CORES == 0
    e_core = e_total // N_CORES
    caps_cols = DEFAULT_CAPS_COLS

    in_maps, slot_maps = make_in_maps(
        Y, etas, src, dst, n, e_core, caps_cols, N_CORES
    )

    key = (n, d, caps_cols)
    if key not in _NC_CACHE:
        _NC_CACHE[key] = build_nc(n, d, caps_cols, 8, N_CORES)
    nc = _NC_CACHE[key]

    res = run_bass_kernel_spmd(
        nc, in_maps, core_ids=list(range(N_CORES)), trace=_trace
    )
    LAST_EXEC_NS = res.exec_time_ns

    w = np.empty(e_total, np.float32)
    for c in range(N_CORES):
        w_tok = res.results[c]["w_out"].reshape(128, -1).T.reshape(-1)
        wslot = slot_maps[c]
        m = wslot >= 0
        w[c * e_core + wslot[m]] = w_tok[m]
    deg = res.results[0]["deg_out"].reshape(-1).astype(np.float32)
    return w, deg
